# revision 1
# baseline (speedup 1.0000x reference)
"""Trainium2 Bass kernel for nn_MoLE (moe_routing).

Reference computation (TOPK=1, so softmax over selected scores == 1.0):
  out[:, 0:32]  = sigmoid(conv2(relu(conv1(rgb_local, Wsel_rgb)))) +
                  sigmoid(conv2(relu(conv1(ir_local,  Wsel_ir))))
  out[:, 32:64] = rgb_dense
  out[:, 64:96] = ir_dense
where Wsel per sample/branch is the argmax-gated expert's conv params.

Sharding: rows (H) split across 8 cores, 60 rows each. Gating (argmax of a
2-way score per sample/branch) is computed on host; selected expert conv
params are passed per-unit to the device. All heavy lifting (3x3 convs,
activations, dense pass-through) runs on the NeuronCores in bf16 compute /
fp32 I/O.

Per-core geometry:
  strip: 60 output rows. Input x padded on host: 65 rows x 642 cols
  (reflect rows/cols baked in). 8 units = 4 samples x 2 branches.
  Each unit processed in 4 "quarters" of 15 output rows; each quarter in
  4 sub-blocks of 4 rows mapped onto the 4 PE column groups.
"""
import os
import sys

sys.path.insert(0, "/opt/trn_rl_repo")
sys.path.insert(0, os.path.dirname(os.path.abspath(__file__)))

import numpy as np
import ml_dtypes

import concourse.bass as bass
import concourse.mybir as mybir
from concourse import tile
from concourse.tile import add_dep_helper
from concourse.bass_utils import run_bass_kernel_spmd

try:
    from tile_fix import split_waits
except ImportError:
    # self-contained fallback (kernel.py must run standalone)
    def split_waits(nc, max_waits=1):
        for bb in nc.main_func.blocks:
            new_insts = []
            for ins in bb.instructions:
                si = ins.sync_info
                if si is not None and si.on_wait and len(si.on_wait) > max_waits:
                    waits = list(si.on_wait)
                    extra, keep = waits[:-max_waits], waits[-max_waits:]
                    for i in range(0, len(extra), max_waits):
                        nop = mybir.InstNoOp(
                            name=nc.get_next_instruction_name(),
                            engine=ins.engine,
                            ins=[], outs=[],
                            sync_info=mybir.SyncInfo(
                                on_wait=extra[i:i + max_waits], on_update=[]),
                            bass_nofuse=True,
                        )
                        new_insts.append(nop)
                    si.on_wait = keep
                new_insts.append(ins)
            bb.instructions[:] = new_insts
        return nc


def merge_pe_incs(nc):
    """Drop the per-matmul semaphore increments on non-stop matmuls (Tile
    emits one on EVERY matmul; the EVT_SEM writes serialize the PE). Only
    each accumulation group's stop matmul keeps its increment; all waits on
    those sems (in every block) are rewritten to the thinner counting,
    rounding up to the next kept increment when gating on a dropped one."""
    all_insts = []
    for bb in nc.main_func.blocks:
        all_insts.extend(bb.instructions)
    mm_sems = set()
    for ins in all_insts:
        if type(ins).__name__ == "InstMatmult" and ins.sync_info:
            for u in (ins.sync_info.on_update or []):
                if u.update_mode == "sem-inc" and u.update_reg is None:
                    mm_sems.add(u.id)
    if not mm_sems:
        return nc
    o2n = {sid: {0: 0} for sid in mm_sems}
    oc = {sid: 0 for sid in mm_sems}
    ncnt = {sid: 0 for sid in mm_sems}
    for ins in all_insts:
        si = ins.sync_info
        if si is None or not si.on_update:
            continue
        drop = (type(ins).__name__ == "InstMatmult"
                and not ins.stop_tensor_calc)
        keep = []
        for u in si.on_update:
            if (u.id in mm_sems and u.update_mode == "sem-inc"
                    and u.update_reg is None):
                oc[u.id] += u.update_value
                if drop:
                    o2n[u.id][oc[u.id]] = ncnt[u.id] + 1
                else:
                    ncnt[u.id] += u.update_value
                    o2n[u.id][oc[u.id]] = ncnt[u.id]
                    keep.append(u)
            else:
                keep.append(u)
        si.on_update = keep
    for ins in all_insts:
        si = ins.sync_info
        if si is None or not si.on_wait:
            continue
        for wt in si.on_wait:
            if wt.id in mm_sems and wt.wait_mode == "sem-ge-imm" \
                    and wt.wait_value is not None:
                wt.wait_value = min(
                    o2n[wt.id].get(wt.wait_value, ncnt[wt.id]),
                    ncnt[wt.id])
    return nc


BF16 = mybir.dt.bfloat16
F32 = mybir.dt.float32

B, C, H, W, E = 4, 32, 480, 640, 4
CH = 16              # hidden channels (C//2)
N_CORES = 8
ROWS = H // N_CORES  # 60 output rows per core
WP = W + 2           # padded row length 642
XROWS = ROWS + 5     # 65 padded input rows per core strip
NQ = 4               # quarters per unit
QR = 15              # output rows per quarter
NS = 4               # sub-blocks per quarter
SR = 4               # conv2 rows per sub-block (incl. 1 junk row in s=3)
C1R = 6              # conv1 rows per sub-block
C1LEN = C1R * WP     # 3852 conv1 elems per sub-block
C2LEN = SR * WP      # 2568 conv2 elems per sub-block
XJROWS = 20          # x rows per quarter job
XJ = XJROWS * WP + 2  # x job tile free size (1-elem guards both ends)
C1CH = [482] * 7 + [478]          # conv1 chunk lengths (sum 3852)
C1OFF = [482 * i for i in range(8)]
HLEN = 321           # conv2 half-row chunk
TAPS = [(dy, dx) for dy in (-1, 0, 1) for dx in (-1, 0, 1)]

N_WAVES = int(os.environ.get("MOLE_WAVES", "8"))


def build_kernel(n_waves=N_WAVES):
    nc = bass.Bass()
    xpad = nc.declare_dram_parameter("xpad", [B, 2, C, XROWS, WP], F32, isOutput=False)
    rgbd = nc.declare_dram_parameter("rgbd", [B, C, ROWS, W], F32, isOutput=False)
    ird = nc.declare_dram_parameter("ird", [B, C, ROWS, W], F32, isOutput=False)
    w1s = nc.declare_dram_parameter("w1s", [4, C, 2 * 9 * CH], BF16, isOutput=False)
    w2s = nc.declare_dram_parameter("w2s", [4, CH, 8 * 9 * C], BF16, isOutput=False)
    b1r = nc.declare_dram_parameter("b1r", [128, 8], F32, isOutput=False)
    b2r = nc.declare_dram_parameter("b2r", [128, 8], F32, isOutput=False)
    emask = nc.declare_dram_parameter("emask", [128, 2], F32, isOutput=False)
    out = nc.declare_dram_parameter("out", [B, 3 * C, ROWS, W], F32, isOutput=True)

    from contextlib import ExitStack
    with tile.TileContext(nc) as tc, ExitStack() as es:
        wpool = es.enter_context(tc.tile_pool(name="wpool", bufs=1))
        xpool = es.enter_context(tc.tile_pool(name="xpool", bufs=2))
        rpool = es.enter_context(tc.tile_pool(name="rpool", bufs=1))
        ypool = es.enter_context(tc.tile_pool(name="ypool", bufs=2))
        opool = es.enter_context(tc.tile_pool(name="opool", bufs=2))
        dpool = es.enter_context(tc.tile_pool(name="dpool", bufs=2))
        tpsum = es.enter_context(tc.tile_pool(name="tpsum", bufs=1, space="PSUM"))
        upsum = es.enter_context(tc.tile_pool(name="upsum", bufs=1, space="PSUM"))

        # --- preload weights/biases ---
        w1t = wpool.tile([128, 2 * 9 * CH], BF16, tag="w1")
        w2t = wpool.tile([128, 8 * 9 * C], BF16, tag="w2")
        b1t = wpool.tile([128, 8], F32, tag="b1")
        b2t = wpool.tile([128, 8], F32, tag="b2")
        for g in range(4):
            nc.sync.dma_start(w1t[32 * g:32 * g + 32, :], w1s[g])
            nc.sync.dma_start(w2t[32 * g:32 * g + CH, :], w2s[g])
        nc.sync.dma_start(b1t[:, :], b1r[:, :])
        nc.sync.dma_start(b2t[:, :], b2r[:, :])
        emt = wpool.tile([128, 2], F32, tag="em")
        nc.sync.dma_start(emt[:, :], emask[:, :])

        prev_mm = None

        # dense pass-through schedule: one (branch, sample) per wave
        dense_jobs = [(br, b) for br in range(2) for b in range(B)]
        DCH = 3200  # dense chunk free size => [128, 3200] fp32, 1.6 MB

        def emit_dense(job_idx):
            br, b = dense_jobs[job_idx]
            src = (rgbd if br == 0 else ird)[b].flatten().rearrange(
                "(p f) -> p f", p=128)
            dst = out[b, (1 + br) * C:(2 + br) * C].flatten().rearrange(
                "(p f) -> p f", p=128)
            F = src.shape[1]  # 9600
            for c0 in range(0, F, DCH):
                ln = min(DCH, F - c0)
                dt_ = dpool.tile([128, DCH], F32, tag="dense")
                nc.sync.dma_start(dt_[:, :ln], src[:, c0:c0 + ln])
                nc.sync.dma_start(dst[:, c0:c0 + ln], dt_[:, :ln])

        for w in range(n_waves):
            p, q = divmod(w, NQ)
            # slot g -> (sample, branch); pairs (0,1) and (2,3)
            units = [(2 * p, 0), (2 * p, 1), (2 * p + 1, 0), (2 * p + 1, 1)]
            uidx = [b * 2 + br for (b, br) in units]

            # --- x loads (fp32 DRAM -> bf16 SBUF, SWDGE cast) ---
            xt = xpool.tile([128, XJ], BF16, tag="x")
            for g, (b, br) in enumerate(units):
                src = xpad[b, br, :, QR * q:QR * q + XJROWS, :]
                nc.gpsimd.dma_start(
                    xt[32 * g:32 * g + 32, 1:1 + XJROWS * WP],
                    src.rearrange("c r w -> c (r w)"))

            # --- conv1 ---
            rt = [rpool.tile([128, C1LEN + 4], BF16, tag=f"r{g}", name=f"rt{g}") for g in range(4)]
            for cc in range(8):
                ln = C1CH[cc]
                o1 = C1OFF[cc]
                T = [tpsum.tile([128, 512], F32, tag=f"t{g}", name=f"T{g}") for g in range(4)]
                for t, (dy, dx) in enumerate(TAPS):
                    for pp in range(16):
                        g = pp % 4
                        s = (pp % 4 + pp // 4) % 4
                        lhs = w1t[32 * g:32 * g + 32,
                                  ((p * 9 + t) * CH):((p * 9 + t) * CH) + CH]
                        base = 1 + (4 * s + 1 + dy) * WP + dx
                        mm = nc.tensor.matmul(
                            T[g][32 * s:32 * s + CH, 0:ln],
                            lhs,
                            xt[32 * g:32 * g + 32, base + o1:base + o1 + ln],
                            start=(t == 0), stop=(t == 8),
                            tile_position=(32 * g, 32 * s),
                        )
                        if prev_mm is not None:
                            add_dep_helper(mm.ins, prev_mm.ins, sync=False,
                                           reason="pe-order")
                        prev_mm = mm
                for g in range(4):
                    nc.scalar.activation(
                        rt[g][:, 1 + o1:1 + o1 + ln], T[g][:, 0:ln],
                        mybir.ActivationFunctionType.Relu,
                        bias=b1t[:, uidx[g]:uidx[g] + 1],
                    )

            # Global-image row reflection of the conv1 feature map: at the
            # top (core 0, quarter 0) conv1 "row -1" must equal conv1 row +1;
            # at the bottom (core 7, quarter 3) conv1 row 60 must equal row
            # 58. Data-driven per-core mask keeps the program SPMD-uniform:
            # row_bad += mask * (row_good - row_bad).
            if q == 0 or q == NQ - 1:
                pbase = 0 if q == 0 else 96
                rbad = 0 if q == 0 else 4
                mcol = 0 if q == 0 else 1
                for g in range(4):
                    bad = rt[g][pbase:pbase + CH, 1 + rbad * WP:1 + (rbad + 1) * WP]
                    good = rt[g][pbase:pbase + CH, 1 + 2 * WP:1 + 3 * WP]
                    etmp = wpool.tile([128, WP], BF16, tag="etmp",
                                      name=f"etmp{w}_{g}", bufs=2)
                    nc.vector.tensor_sub(etmp[pbase:pbase + CH, :], good, bad)
                    nc.vector.scalar_tensor_tensor(
                        out=bad, in0=etmp[pbase:pbase + CH, :],
                        scalar=emt[pbase:pbase + CH, mcol:mcol + 1],
                        in1=bad,
                        op0=mybir.AluOpType.mult, op1=mybir.AluOpType.add)

            # conv1 feature-map reflect padding: junk cols pc=0/641 must hold
            # the reflect values (pc=2 / pc=639) since conv2's dx=+-1 taps at
            # valid edge columns read them.
            for g in range(4):
                nc.vector.tensor_copy(
                    rt[g][:, 1:1 + 5 * WP + 1:WP],
                    rt[g][:, 3:3 + 5 * WP + 1:WP])
                nc.vector.tensor_copy(
                    rt[g][:, WP:6 * WP + 1:WP],
                    rt[g][:, W:W + 5 * WP + 1:WP])

            # --- conv2 ---
            yt = [ypool.tile([128, C2LEN], BF16, tag=f"y{g}", name=f"yt{g}") for g in range(4)]
            for lr2 in range(4):
                for hh in range(2):
                    o2 = lr2 * WP + HLEN * hh
                    U = [upsum.tile([128, 512], F32, tag=f"u{g}", name=f"U{g}") for g in range(4)]
                    for t, (dy, dx) in enumerate(TAPS):
                        base2 = 1 + (1 + dy) * WP + dx
                        for pp in range(16):
                            s = pp % 4
                            g = (pp % 4 + pp // 4) % 4
                            k = (s + 2 * hh) % 4
                            mm = nc.tensor.matmul(
                                U[g][32 * k:32 * k + 32, 0:HLEN],
                                w2t[32 * s:32 * s + CH,
                                    (uidx[g] * 9 + t) * C:
                                    (uidx[g] * 9 + t) * C + C],
                                rt[g][32 * s:32 * s + CH,
                                      base2 + o2:base2 + o2 + HLEN],
                                start=(t == 0), stop=(t == 8),
                                tile_position=(32 * s, 32 * k),
                            )
                            if prev_mm is not None:
                                add_dep_helper(mm.ins, prev_mm.ins, sync=False,
                                               reason="pe-order")
                            prev_mm = mm
                    for g in range(4):
                        nc.scalar.activation(
                            yt[g][:, o2:o2 + HLEN], U[g][:, 0:HLEN],
                            mybir.ActivationFunctionType.Sigmoid,
                            bias=b2t[:, uidx[g]:uidx[g] + 1],
                        )

            # --- pair add + output DMA ---
            for pair in range(2):
                b = units[2 * pair][0]
                ot = opool.tile([128, C2LEN], F32, tag=f"o{pair}")
                nc.vector.tensor_tensor(
                    out=ot[:, :], in0=yt[2 * pair][:, :], in1=yt[2 * pair + 1][:, :],
                    op=mybir.AluOpType.add)
                for s in range(4):
                    nrows = 3 if s == 3 else 4
                    for hh in range(2):
                        k = (s + 2 * hh) % 4
                        src = ot[32 * k:32 * k + 32, :].rearrange(
                            "c (r w) -> c r w", r=SR)
                        nc.sync.dma_start(
                            out[b, 0:C, QR * q + SR * s:QR * q + SR * s + nrows,
                                320 * hh:320 * hh + 320],
                            src[:, 0:nrows, 321 * hh + (1 - hh):
                                321 * hh + (1 - hh) + 320])

            if w < len(dense_jobs):
                emit_dense(w)

        # if fewer waves than dense jobs, emit the rest
        for j in range(n_waves, len(dense_jobs)):
            emit_dense(j)

    merge_pe_incs(nc)
    split_waits(nc)
    return nc


def _host_gate_and_pack(inputs):
    """Host-side gating (argmax over 2 scores per sample/branch) and packing
    of selected expert conv params into device layouts."""
    rl = inputs["rgb_local"]
    il = inputs["ir_local"]
    sc_rgb = rl.reshape(B, -1) @ inputs["gate_rgb_w"].reshape(2, -1).T \
        + inputs["gate_rgb_b"]
    sc_ir = il.reshape(B, -1) @ inputs["gate_ir_w"].reshape(2, -1).T \
        + inputs["gate_ir_b"]
    e_rgb = np.argmax(sc_rgb, axis=1)          # in {0,1}
    e_ir = np.argmax(sc_ir, axis=1) + 2        # in {2,3}

    ew1, eb1 = inputs["ew1"], inputs["eb1"]    # [E,16,32,3,3], [E,16]
    ew2, eb2 = inputs["ew2"], inputs["eb2"]    # [E,32,16,3,3], [E,32]

    eu = np.empty(8, np.int64)                 # unit u = b*2+br
    for b in range(B):
        eu[b * 2 + 0] = e_rgb[b]
        eu[b * 2 + 1] = e_ir[b]

    # w1[g, ci, p, tap, co] = ew1[eu(u(g,p)), co, ci, tap]
    w1 = np.empty((4, C, 2, 9, CH), np.float32)
    for g in range(4):
        for p in range(2):
            u = (2 * p + g // 2) * 2 + (g % 2)
            sel = ew1[eu[u]]                   # [16, 32, 3, 3]
            w1[g, :, p] = np.moveaxis(sel.reshape(CH, C, 9), [0, 1, 2], [2, 0, 1])

    w2 = np.empty((4, CH, 8, 9, C), np.float32)
    for u in range(8):
        sel = ew2[eu[u]]                       # [32, 16, 3, 3]
        m = np.moveaxis(sel.reshape(C, CH, 9), [0, 1, 2], [2, 0, 1])  # [16,9,32]
        for s in range(4):
            w2[s, :, u] = m

    b1 = np.zeros((128, 8), np.float32)
    b2 = np.zeros((128, 8), np.float32)
    for u in range(8):
        for s in range(4):
            b1[32 * s:32 * s + CH, u] = eb1[eu[u]]
            b2[32 * s:32 * s + C, u] = eb2[eu[u]]

    return (w1.reshape(4, C, 2 * 9 * CH).astype(ml_dtypes.bfloat16),
            w2.reshape(4, CH, 8 * 9 * C).astype(ml_dtypes.bfloat16),
            b1, b2)


def _build_xpad(x, core):
    """[B, C, H, W] fp32 -> padded strip [B, C, XROWS, WP] for one core."""
    r0 = ROWS * core - 2
    rows = np.arange(r0, r0 + XROWS)
    rows = np.where(rows < 0, -rows, rows)
    rows = np.where(rows >= H, 2 * (H - 1) - rows, rows)
    strip = x[:, :, rows, :]                       # [B, C, XROWS, W]
    padded = np.empty((B, C, XROWS, WP), np.float32)
    padded[:, :, :, 1:W + 1] = strip
    padded[:, :, :, 0] = strip[:, :, :, 1]
    padded[:, :, :, W + 1] = strip[:, :, :, W - 2]
    return padded


_CACHE = {}


def _get_nc():
    if "nc" not in _CACHE:
        _CACHE["nc"] = build_kernel()
    return _CACHE["nc"]


def make_in_maps(inputs):
    w1, w2, b1, b2 = _host_gate_and_pack(inputs)
    xp_rgb = inputs["rgb_local"]
    xp_ir = inputs["ir_local"]
    in_maps = []
    for core in range(N_CORES):
        em = np.zeros((128, 2), np.float32)
        if core == 0:
            em[0:CH, 0] = 1.0
        if core == N_CORES - 1:
            em[96:96 + CH, 1] = 1.0
        xpad = np.stack(
            [_build_xpad(xp_rgb, core), _build_xpad(xp_ir, core)], axis=1)
        in_maps.append(dict(
            xpad=np.ascontiguousarray(xpad), emask=em,
            rgbd=np.ascontiguousarray(
                inputs["rgb_dense"][:, :, ROWS * core:ROWS * (core + 1), :]),
            ird=np.ascontiguousarray(
                inputs["ir_dense"][:, :, ROWS * core:ROWS * (core + 1), :]),
            w1s=w1, w2s=w2, b1r=b1, b2r=b2,
        ))
    return in_maps


def kernel(**inputs):
    inputs = {k: np.asarray(v) for k, v in inputs.items()}
    nc = _get_nc()
    in_maps = make_in_maps(inputs)
    res = run_bass_kernel_spmd(nc, in_maps, list(range(N_CORES)))
    strips = [res.results[i]["out"] for i in range(N_CORES)]
    return np.concatenate(strips, axis=2)


if __name__ == "__main__":
    rng = np.random.default_rng(0)
    fake = dict(
        rgb_local=rng.standard_normal((B, C, H, W), dtype=np.float32),
        ir_local=rng.standard_normal((B, C, H, W), dtype=np.float32),
        rgb_dense=rng.standard_normal((B, C, H, W), dtype=np.float32),
        ir_dense=rng.standard_normal((B, C, H, W), dtype=np.float32),
        gate_rgb_w=rng.standard_normal((2, C * H * W), dtype=np.float32) * 1e-3,
        gate_rgb_b=rng.standard_normal(2).astype(np.float32),
        gate_ir_w=rng.standard_normal((2, C * H * W), dtype=np.float32) * 1e-3,
        gate_ir_b=rng.standard_normal(2).astype(np.float32),
        ew1=rng.standard_normal((E, CH, C, 3, 3), dtype=np.float32) * 0.05,
        eb1=rng.standard_normal((E, CH)).astype(np.float32),
        ew2=rng.standard_normal((E, C, CH, 3, 3), dtype=np.float32) * 0.05,
        eb2=rng.standard_normal((E, C)).astype(np.float32),
    )
    o = kernel(**fake)
    print("out shape:", o.shape)

